# revision 1
# baseline (speedup 1.0000x reference)
"""DiscreteMMSE Trainium2 Bass kernel.

Math (per batch row b):
  Z = data[b] @ W                      [N, T]   (W = squeeze(task_pool).T)
  resid = Z - targets[b][:, None]      [N, T]
  S'[i] = sum_{n<i} resid[n]^2         (strict cumsum over N; S'[0] = 0)
  E = exp(-0.5*S' - max_t(-0.5*S'))    (exact softmax-stable weights)
  out[b, i] = targets[b, i] + (sum_t E[i]*resid[i]) / (sum_t E[i])

Identical to the reference softmax-posterior MMSE prediction: the Gaussian
log-pdf constant and common shifts cancel in the softmax, and
pred = sum_t post*Z[i] = targets[i] - sum_t post*(targets[i]-Z[i]) collapses
onto resid. Row 0 (uniform prior over tasks) falls out of the strict cumsum.

Layout per NeuronCore (pure data parallel over B: 8 rows each, no collectives):
  - N=256 rows on partitions as two 128-row chunks; T=4096 on the free dim.
  - float32r (TF32-like, fp32 with 12 low mantissa bits dropped) matmuls are
    measured EXACT on f32r inputs and run 4x faster than fp32 matmuls, so
    every fp32 operand is split hi+lo into two f32r planes, making every
    matmul here fp32-exact at f32r speed:
      resid: lhsT = [data.T; targets] and rhs = [W; -1] each split hi/lo,
             3-term product (the lo*lo term is below fp32 resolution).
      cumsum input sq = resid^2: hi = f32r(sq) (GpSimd cast of the ScalarE
             Square output), lo = f32r(sq - hi) (GpSimd/VectorE split).
  - strict cumsum over N via triangular-ones f32r matmuls on TensorE:
    chunk0: U.T@{hi0,lo0} ; chunk1: U.T@{hi1,lo1} + ones.T@{hi0,lo0},
    accumulated in one PSUM group.
  - PSUM evacuation fused with the row-max: tensor_scalar(mult -0.5,
    accum max) on VectorE.
  - Exp on ScalarE with per-partition bias = -rowmax, accum_out = denominator.
  - numerator: resid recomputed (hi-term only, benign) into PSUM, E*=resid
    in place on VectorE, row-sum via in-place ScalarE Copy accum.
  - modulo-scheduled emission: engines execute their instruction streams
    IN ORDER, so per-jt rounds interleave batch b's stage-1 chain
    (resid->sq->hi/lo->cumsum->evac) with batch b-1's stage-2 chain
    (exp->recompute->mul->numsum); this keeps ready work at the front of
    every engine queue and hides all cross-engine round-trips.
"""

import numpy as np

B, N, D, T = 64, 256, 64, 4096
NCORES = 8
BPC = B // NCORES  # batch rows per core
NCH = 2            # partition chunks of N
PB = 128           # partitions per chunk
PT = 1024          # psum tile free size (2 banks)
MT = 512           # matmul moving free size (1 bank)
NJT = T // PT      # psum tiles per chunk row
NMM = PT // MT     # matmuls per psum tile

_cached_nc = None


def _build():
    import concourse.bacc as bacc
    import concourse.mybir as mybir
    import concourse.tile as tile
    from concourse import masks

    F32 = mybir.dt.float32
    F32R = mybir.dt.float32r
    AF = mybir.ActivationFunctionType
    OP = mybir.AluOpType

    nc = bacc.Bacc("TRN2", debug=False)
    data_d = nc.dram_tensor("data", (BPC, N, D), F32, kind="ExternalInput")
    targ_d = nc.dram_tensor("targets", (BPC, N), F32, kind="ExternalInput")
    pool_d = nc.dram_tensor("task_pool", (T, D), F32, kind="ExternalInput")
    out_d = nc.dram_tensor("out", (BPC, N), F32, kind="ExternalOutput")

    with tile.TileContext(nc) as tc:
        with tc.tile_pool(name="const", bufs=1) as const:
            utri = const.tile([PB, PB], F32R)     # strictly-upper ones (lhsT)
            onesm = const.tile([PB, PB], F32R)    # all-ones
            waug_h = const.tile([D + 1, T], F32R)       # f32r hi of [W ; -1]
            waug_l = const.tile([D + 1, T], F32R)       # f32r lo
            daug_h = const.tile([D + 1, BPC * N], F32R)  # hi of [data.T ; tgt]
            daug_l = const.tile([D + 1, BPC * N], F32R)  # lo
            tpart = [const.tile([PB, BPC], F32, name=f"tpart{c}", tag=f"tpart{c}") for c in range(NCH)]
            den = [const.tile([PB, BPC], F32, name=f"den{c}", tag=f"den{c}") for c in range(NCH)]
            num = [const.tile([PB, BPC], F32, name=f"num{c}", tag=f"num{c}") for c in range(NCH)]

            nc.any.memset(onesm[:].bitcast(F32), 1.0)

            # ---- setup: transpose task_pool and data into lhsT layouts ----
            with (
                tc.tile_pool(name="ld", bufs=1) as ld,
                tc.tile_pool(name="tps", bufs=4, space="PSUM") as tps,
            ):
                ident = ld.tile([PB, PB], F32, tag="ident", name="ident")
                masks.make_identity(nc, ident[:])
                utri_f = ld.tile([PB, PB], F32, tag="utri_f", name="utri_f")
                masks.make_upper_triangular(nc, utri_f[:], 1.0, diag=False)
                nc.vector.tensor_copy(utri[:], utri_f[:])
                waug = ld.tile([D + 1, T], F32, tag="waug", name="waug")
                daug = ld.tile([D + 1, BPC * N], F32, tag="daug", name="daug")
                nc.any.memset(waug[D : D + 1, :], -1.0)
                wbig = ld.tile([PB, (T // PB) * D], F32, tag="wbig", name="wbig")
                nc.sync.dma_start(
                    wbig[:].rearrange("p (k d) -> p k d", d=D),
                    pool_d[:].rearrange("(k p) d -> p k d", p=PB),
                )
                for k in range(T // PB):
                    pt = tps.tile([D, PB], F32, tag="pt", name="pt")
                    nc.tensor.transpose(pt[:], wbig[:, k * D : (k + 1) * D], ident[:])
                    nc.vector.tensor_copy(waug[0:D, k * PB : (k + 1) * PB], pt[:])
                for b in range(BPC):
                    nc.sync.dma_start(
                        daug[D : D + 1, b * N : (b + 1) * N], targ_d[b : b + 1, :]
                    )
                    dbig = ld.tile([PB, NCH * D], F32, tag=f"dbig{b % 2}", name="dbig")
                    nc.sync.dma_start(
                        dbig[:].rearrange("p (c d) -> p c d", d=D),
                        data_d[b].rearrange("(c p) d -> p c d", p=PB),
                    )
                    for c in range(NCH):
                        pt = tps.tile([D, PB], F32, tag="pt", name="pt")
                        nc.tensor.transpose(
                            pt[:], dbig[:, c * D : (c + 1) * D], ident[:]
                        )
                        nc.vector.tensor_copy(
                            daug[0:D, b * N + c * PB : b * N + (c + 1) * PB], pt[:]
                        )
                        tv = targ_d[b, c * PB : (c + 1) * PB].rearrange(
                            "(p one) -> p one", one=1
                        )
                        nc.sync.dma_start(tpart[c][:, b : b + 1], tv)
                nc.vector.tensor_copy(waug_h[:], waug[:])
                nc.vector.tensor_sub(waug_l[:], waug[:], waug_h[:].bitcast(F32))
                nc.vector.tensor_copy(daug_h[:], daug[:])
                nc.vector.tensor_sub(daug_l[:], daug[:], daug_h[:].bitcast(F32))

            # ---- main pipeline ----
            with (
                tc.tile_pool(name="sq32p", bufs=3) as sq32p,
                tc.tile_pool(name="hilo", bufs=2) as hilo,
                tc.tile_pool(name="avp", bufs=2) as avp,
                tc.tile_pool(name="ep", bufs=3) as ep,
                tc.tile_pool(name="small", bufs=4) as small,
                tc.tile_pool(name="rpp", bufs=2, space="PSUM") as rpp,
                tc.tile_pool(name="spp", bufs=2, space="PSUM") as spp,
            ):

                def s1_alloc(b):
                    av = [
                        avp.tile([PB, T], F32, tag=f"av{c}", name=f"av{c}")
                        for c in range(NCH)
                    ]
                    mx2 = [
                        small.tile([PB, NJT], F32, tag=f"mx2{c}", name=f"mx2{c}")
                        for c in range(NCH)
                    ]
                    return av, mx2

                def s1_round(b, jt, av, mx2):
                    """per-jt chain: resid -> sq -> hi/lo -> cumsum -> evac."""
                    js = slice(jt * PT, (jt + 1) * PT)
                    his, los = [], []
                    for c in range(NCH):
                        cs = slice(b * N + c * PB, b * N + (c + 1) * PB)
                        rp = rpp.tile([PB, PT], F32, tag="rp", name="rp")
                        for h in range(NMM):
                            lo_ = jt * PT + h * MT
                            wsl = slice(lo_, lo_ + MT)
                            osl = rp[:, h * MT : (h + 1) * MT]
                            nc.tensor.matmul(
                                osl, daug_h[:, cs], waug_h[:, wsl],
                                start=True, stop=False,
                            )
                            nc.tensor.matmul(
                                osl, daug_h[:, cs], waug_l[:, wsl],
                                start=False, stop=False,
                            )
                            nc.tensor.matmul(
                                osl, daug_l[:, cs], waug_h[:, wsl],
                                start=False, stop=True,
                            )
                        sq32 = sq32p.tile([PB, PT], F32, tag="sq32", name="sq32")
                        nc.scalar.activation(sq32[:], rp[:], AF.Square)
                        hi_t = hilo.tile([PB, PT], F32R, tag=f"hi{c}", name=f"hi{c}")
                        nc.gpsimd.tensor_copy(hi_t[:], sq32[:])
                        lo_t = hilo.tile([PB, PT], F32R, tag=f"lo{c}", name=f"lo{c}")
                        if c == 0 or jt <= 1:
                            nc.gpsimd.tensor_sub(
                                lo_t[:], sq32[:], hi_t[:].bitcast(F32)
                            )
                        else:
                            nc.vector.tensor_sub(
                                lo_t[:], sq32[:], hi_t[:].bitcast(F32)
                            )
                        his.append(hi_t)
                        los.append(lo_t)
                    for c in range(NCH):
                        sp = spp.tile([PB, PT], F32, tag="sp", name="sp")
                        for h in range(NMM):
                            hsl = slice(h * MT, (h + 1) * MT)
                            ssl = sp[:, hsl]
                            nc.tensor.matmul(
                                ssl, utri[:], his[c][:, hsl],
                                start=True, stop=False,
                            )
                            nc.tensor.matmul(
                                ssl, utri[:], los[c][:, hsl],
                                start=False, stop=(c == 0),
                            )
                            if c == 1:
                                nc.tensor.matmul(
                                    ssl, onesm[:], his[0][:, hsl],
                                    start=False, stop=False,
                                )
                                nc.tensor.matmul(
                                    ssl, onesm[:], los[0][:, hsl],
                                    start=False, stop=True,
                                )
                        nc.vector.tensor_scalar(
                            out=av[c][:, js],
                            in0=sp[:],
                            scalar1=-0.5,
                            scalar2=None,
                            op0=OP.mult,
                            op1=OP.max,
                            accum_out=mx2[c][:, jt : jt + 1],
                        )

                def s1_finish(b, mx2):
                    """negated row-max once all evac partials of b landed."""
                    nbs = []
                    for c in range(NCH):
                        nb = small.tile([PB, 1], F32, tag=f"nb{c}", name=f"nb{c}")
                        nc.vector.tensor_reduce(
                            nb[:], mx2[c][:], axis=mybir.AxisListType.X, op=OP.max,
                            negate=True,
                        )
                        nbs.append(nb)
                    return nbs

                def s2_alloc(b):
                    den4 = [
                        small.tile([PB, NJT], F32, tag=f"den4{c}", name=f"den4{c}")
                        for c in range(NCH)
                    ]
                    num4 = [
                        small.tile([PB, NJT], F32, tag=f"num4{c}", name=f"num4{c}")
                        for c in range(NCH)
                    ]
                    return den4, num4

                def s2_round(b, jt, av, nbs, den4, num4):
                    """exp -> resid recompute -> E*resid -> numsum for (b, jt)."""
                    js = slice(jt * PT, (jt + 1) * PT)
                    for c in range(NCH):
                        ev = ep.tile([PB, PT], F32, tag=f"E{c}", name=f"E{c}")
                        nc.scalar.activation(
                            ev[:],
                            av[c][:, js],
                            AF.Exp,
                            bias=nbs[c][:],
                            scale=1.0,
                            accum_out=den4[c][:, jt : jt + 1],
                        )
                        lhsT_r = daug_h[:, b * N + c * PB : b * N + (c + 1) * PB]
                        rp2 = spp.tile([PB, PT], F32, tag="sp", name="rp2")
                        for h in range(NMM):
                            lo_ = jt * PT + h * MT
                            nc.tensor.matmul(
                                rp2[:, h * MT : (h + 1) * MT],
                                lhsT_r,
                                waug_h[:, lo_ : lo_ + MT],
                            )
                        nc.vector.tensor_mul(ev[:], ev[:], rp2[:])
                        nc.vector.tensor_scalar(
                            out=ev[:],
                            in0=ev[:],
                            scalar1=1.0,
                            scalar2=None,
                            op0=OP.mult,
                            op1=OP.add,
                            accum_out=num4[c][:, jt : jt + 1],
                        )

                def s2_finish(b, den4, num4):
                    for c in range(NCH):
                        nc.vector.tensor_reduce(
                            den[c][:, b : b + 1], den4[c][:],
                            axis=mybir.AxisListType.X, op=OP.add,
                        )
                        nc.vector.tensor_reduce(
                            num[c][:, b : b + 1], num4[c][:],
                            axis=mybir.AxisListType.X, op=OP.add,
                        )

                # modulo-scheduled pipeline: per-jt rounds interleave batch b's
                # stage-1 chain with batch b-1's stage-2 chain so each engine's
                # in-order stream always has ready work at the front.
                prev = None
                for b in range(BPC):
                    av, mx2 = s1_alloc(b)
                    if prev is not None:
                        pb, pav, pnbs, pden4, pnum4 = prev
                    for jt in range(NJT):
                        if prev is not None:
                            s2_round(pb, jt, pav, pnbs, pden4, pnum4)
                        s1_round(b, jt, av, mx2)
                    if prev is not None:
                        s2_finish(pb, pden4, pnum4)
                    nbs = s1_finish(b, mx2)
                    den4, num4 = s2_alloc(b)
                    prev = (b, av, nbs, den4, num4)
                pb, pav, pnbs, pden4, pnum4 = prev
                for jt in range(NJT):
                    s2_round(pb, jt, pav, pnbs, pden4, pnum4)
                s2_finish(pb, pden4, pnum4)

                # finals: out = targets + num/den
                for c in range(NCH):
                    rec = small.tile([PB, BPC], F32, tag=f"rec{c}", name=f"rec{c}")
                    prod = small.tile([PB, BPC], F32, tag=f"prod{c}", name=f"prod{c}")
                    outv = small.tile([PB, BPC], F32, tag=f"outv{c}", name=f"outv{c}")
                    nc.vector.reciprocal(rec[:], den[c][:])
                    nc.vector.tensor_mul(prod[:], num[c][:], rec[:])
                    nc.vector.tensor_add(outv[:], tpart[c][:], prod[:])
                    ov = out_d[:, c * PB : (c + 1) * PB].rearrange("b p -> p b")
                    nc.sync.dma_start(ov, outv[:])

    nc.compile()
    return nc


def _get_nc():
    global _cached_nc
    if _cached_nc is None:
        _cached_nc = _build()
    return _cached_nc


_cached_runner = None


def _get_runner():
    """Build once: a cached jax.jit shard_map over the 8 NeuronCores.

    run_bass_kernel_spmd/run_bass_via_pjrt construct a fresh jax.jit closure
    per call (full retrace); caching the callable keeps repeat calls cheap.
    """
    global _cached_runner
    if _cached_runner is None:
        import jax
        from jax.sharding import Mesh, PartitionSpec
        from concourse import bass2jax
        from concourse.bass2jax import _bass_exec_p, partition_id_tensor
        import concourse.mybir as mybir

        try:
            from jax.experimental.shard_map import shard_map
        except ImportError:
            from jax.shard_map import shard_map

        bass2jax.install_neuronx_cc_hook()
        nc = _get_nc()
        partition_name = (
            nc.partition_id_tensor.name if nc.partition_id_tensor else None
        )
        in_names, out_names, out_avals, zero_outs = [], [], [], []
        for alloc in nc.m.functions[0].allocations:
            if not isinstance(alloc, mybir.MemoryLocationSet):
                continue
            name = alloc.memorylocations[0].name
            if alloc.kind == "ExternalInput":
                if name != partition_name:
                    in_names.append(name)
            elif alloc.kind == "ExternalOutput":
                out_names.append(name)
                shape = tuple(alloc.tensor_shape)
                dtype = mybir.dt.np(alloc.dtype)
                out_avals.append(jax.core.ShapedArray(shape, dtype))
                zero_outs.append(np.zeros((NCORES * shape[0], *shape[1:]), dtype))
        n_params = len(in_names)
        all_names = list(in_names) + list(out_names)
        if partition_name is not None:
            all_names.append(partition_name)
        donate = tuple(range(n_params, n_params + len(out_names)))

        def _body(*args):
            operands = list(args)
            if partition_name is not None:
                operands.append(partition_id_tensor())
            return tuple(
                _bass_exec_p.bind(
                    *operands,
                    out_avals=tuple(out_avals),
                    in_names=tuple(all_names),
                    out_names=tuple(out_names),
                    lowering_input_output_aliases=(),
                    sim_require_finite=True,
                    sim_require_nnan=True,
                    nc=nc,
                )
            )

        devices = jax.devices()[:NCORES]
        mesh = Mesh(np.asarray(devices), ("core",))
        in_specs = tuple(
            PartitionSpec() if name == "task_pool" else PartitionSpec("core")
            for name in in_names
        ) + (PartitionSpec("core"),) * len(out_names)
        sharded = jax.jit(
            shard_map(
                _body,
                mesh=mesh,
                in_specs=in_specs,
                out_specs=(PartitionSpec("core"),) * len(out_names),
                check_rep=False,
            ),
            donate_argnums=donate,
            keep_unused=True,
        )
        _cached_runner = (sharded, in_names, out_names, out_avals, zero_outs)
    return _cached_runner


def _kernel_fallback(data, targets, tp):
    """Robust path via the stock SPMD runner (fresh jit each call)."""
    from concourse.bass_utils import run_bass_kernel_spmd

    nc = _get_nc()
    in_maps = [
        {
            "data": data[i * BPC : (i + 1) * BPC],
            "targets": targets[i * BPC : (i + 1) * BPC],
            "task_pool": tp,
        }
        for i in range(NCORES)
    ]
    res = run_bass_kernel_spmd(nc, in_maps, core_ids=list(range(NCORES)))
    return np.concatenate([r["out"] for r in res.results], axis=0)


def kernel(data, targets, task_pool, **_):
    data = np.ascontiguousarray(np.asarray(data, np.float32))
    targets = np.ascontiguousarray(np.asarray(targets, np.float32))
    tp = np.ascontiguousarray(np.asarray(task_pool, np.float32).reshape(T, D))

    try:
        sharded, in_names, out_names, out_avals, zero_outs = _get_runner()
        full = {
            "data": data.reshape(NCORES * BPC, N, D),
            "targets": targets.reshape(NCORES * BPC, N),
            "task_pool": tp,
        }
        args = [full[name] for name in in_names]
        args += [np.zeros_like(z) for z in zero_outs]
        outs = sharded(*args)
        out = np.asarray(outs[out_names.index("out")])
        return out.reshape(B, N)
    except Exception:
        return _kernel_fallback(data, targets, tp)



# revision 30
# speedup vs baseline: 1.3907x; 1.3907x over previous
"""DiscreteMMSE Trainium2 Bass kernel.

Math (per batch row b):
  Z = data[b] @ W                      [N, T]   (W = squeeze(task_pool).T)
  sq = (Z - targets[b][:, None])^2     [N, T]
  S[i] = sum_{n<i} sq[n]               (strict cumsum over N; S[0] = 0)
  E = exp(-0.5*(S - min_t S))          (softmax-stable weights)
  out[b, i] = targets[b, i] + (sum_t E[i]*(Z-t)[i]) / (sum_t E[i])

Identical to the reference softmax-posterior MMSE prediction: the Gaussian
log-pdf constant and row-wise shifts cancel in the softmax, and
pred = sum_t post*Z = t + sum_t post*(Z-t). Row 0 (uniform prior over
tasks) falls out of the strict cumsum.

Layout per NeuronCore (pure data parallel over B: 8 rows each, no
collectives). N=256 rows on partitions as two 128-row chunks; T=4096 on
the free dim. Engine budget per (b,jt) round is balanced across all four
compute engines:
  - TensorE: Z via ONE 128-contraction f32r matmul per 512-slice with
    lhsT=[d_hi; d_lo] (hi/lo split of data.T) and rhs=[W; W] (full fp32
    bits bitcast to f32r; the PE's 12-bit mantissa drop on read is
    compensated by the lo plane of data, and W's truncation noise is
    ~1e-3 per Z entry -- well within tolerance). Strict cumsum over N via
    triangular-ones f32r matmuls (chunk1 adds ones.T@sq0), reading the
    fp32 sq tile bitcast as f32r (truncation noise ~0.07 nats on logits).
    Z is recomputed for stage 2 (cheaper than buffering it in SBUF).
  - ScalarE (Act): sq = Square(Z + bias) with per-partition bias=-targets
    straight out of PSUM; E = ONE big [128, 4096] Exp per (b,chunk) with
    scale=-0.5, bias=0.5*min_t S, accum_out = denominator. A minority of
    PSUM->SBUF cumsum evacuations also run here (Copy) to offload VectorE.
  - VectorE (DVE): most cumsum evacuations PSUM->SBUF fused with the
    running row-min (tensor_scalar accum min); numerator via ONE fused
    scalar_tensor_tensor: (Z_psum - t)*E with accum_out = running sum.
  - GpSimd (Pool): row-min for the Act-evacuated tiles (SBUF only; GpSimd
    has no PSUM port).
  - modulo-scheduled emission: engines execute their instruction streams
    IN ORDER, so per-jt rounds interleave batch b's stage-1 chain
    (Z->sq->cumsum->evac) with batch b-1's stage-2 chain (Zre->E*(Z-t));
    the big Exp for b runs between the round groups.
"""

import numpy as np

B, N, D, T = 64, 256, 64, 4096
NCORES = 8
BPC = B // NCORES  # batch rows per core
NCH = 2            # partition chunks of N
PB = 128           # partitions per chunk
PT = 1024          # psum tile free size (2 banks)
MT = 512           # matmul moving free size (1 bank)
NJT = T // PT      # psum tiles per chunk row
NMM = PT // MT     # matmuls per psum tile

_cached_nc = None


def _evac_on_act(b, jt, c):
    """Which cumsum evacuations run on ScalarE+GpSimd instead of VectorE.

    ~9 of 64 rebalances DVE (evac+numerator) against Act (square+exp).
    Only early jts: the row-min of late jts gates the exp bias."""
    return c == 0 and (jt == 1 or (jt == 2 and b == 3))


def _build():
    import concourse.bacc as bacc
    import concourse.mybir as mybir
    import concourse.tile as tile
    from concourse import masks

    F32 = mybir.dt.float32
    F32R = mybir.dt.float32r
    AF = mybir.ActivationFunctionType
    OP = mybir.AluOpType

    nc = bacc.Bacc("TRN2", debug=False)
    data_d = nc.dram_tensor("data", (BPC, N, D), F32, kind="ExternalInput")
    targ_d = nc.dram_tensor("targets", (BPC, N), F32, kind="ExternalInput")
    pool_d = nc.dram_tensor("task_pool", (T, D), F32, kind="ExternalInput")
    out_d = nc.dram_tensor("out", (BPC, N), F32, kind="ExternalOutput")

    with tile.TileContext(nc) as tc:
        with tc.tile_pool(name="const", bufs=1) as const:
            utri = const.tile([PB, PB], F32R)     # strictly-upper ones (lhsT)
            onesm = const.tile([PB, PB], F32R)    # all-ones
            wrep = const.tile([PB, T], F32R)      # [W ; W] full fp32 bits
            dstk = const.tile([PB, BPC * N], F32R)  # [data.T hi ; data.T lo]
            tpart = [const.tile([PB, BPC], F32, name=f"tpart{c}", tag=f"tpart{c}") for c in range(NCH)]
            tneg = [const.tile([PB, BPC], F32, name=f"tneg{c}", tag=f"tneg{c}") for c in range(NCH)]
            den = [const.tile([PB, BPC], F32, name=f"den{c}", tag=f"den{c}") for c in range(NCH)]
            num = [const.tile([PB, BPC], F32, name=f"num{c}", tag=f"num{c}") for c in range(NCH)]


            nc.any.memset(onesm[:].bitcast(F32), 1.0)

            # ---- setup: transpose task_pool and data into lhsT layouts ----
            with (
                tc.tile_pool(name="ld", bufs=1) as ld,
                tc.tile_pool(name="tps", bufs=4, space="PSUM") as tps,
            ):
                ident = ld.tile([PB, PB], F32, tag="ident", name="ident")
                masks.make_identity(nc, ident[:])
                utri_f = ld.tile([PB, PB], F32, tag="utri_f", name="utri_f")
                masks.make_upper_triangular(nc, utri_f[:], 1.0, diag=False)
                nc.vector.tensor_copy(utri[:], utri_f[:])
                wbig = ld.tile([PB, (T // PB) * D], F32, tag="wbig", name="wbig")
                NK = T // PB
                KC = NK // 4  # chunk the load so transposes overlap the DMA
                for q in range(4):
                    nc.sync.dma_start(
                        wbig[:, q * KC * D : (q + 1) * KC * D].rearrange(
                            "p (k d) -> p k d", d=D
                        ),
                        pool_d[q * KC * PB : (q + 1) * KC * PB].rearrange(
                            "(k p) d -> p k d", p=PB
                        ),
                    )
                for k in range(NK):
                    pt = tps.tile([D, PB], F32, tag="pt", name="pt")
                    nc.tensor.transpose(pt[:], wbig[:, k * D : (k + 1) * D], ident[:])
                    # split the PSUM->SBUF evacuations across Act and DVE
                    if k % 2 == 0:
                        nc.scalar.activation(
                            wrep[0:D, k * PB : (k + 1) * PB], pt[:], AF.Copy
                        )
                    else:
                        nc.vector.tensor_copy(
                            wrep[0:D, k * PB : (k + 1) * PB], pt[:]
                        )
                # duplicate the f32r-rounded W into the low 64 partitions
                # (GpSimd: SBUF-only copy on the otherwise idle engine)
                nc.gpsimd.tensor_copy(wrep[D : 2 * D, :], wrep[0:D, :])
                for b in range(BPC):
                    dbig = ld.tile([PB, NCH * D], F32, tag=f"dbig{b % 2}", name="dbig")
                    nc.sync.dma_start(
                        dbig[:].rearrange("p (c d) -> p c d", d=D),
                        data_d[b].rearrange("(c p) d -> p c d", p=PB),
                    )
                    for c in range(NCH):
                        cs = slice(b * N + c * PB, b * N + (c + 1) * PB)
                        pt = tps.tile([D, PB], F32, tag="pt", name="pt")
                        nc.tensor.transpose(
                            pt[:], dbig[:, c * D : (c + 1) * D], ident[:]
                        )
                        # hi: f32r-rounding convert copy; lo: exact fp32 rest
                        if c == 0:
                            nc.scalar.activation(dstk[0:D, cs], pt[:], AF.Copy)
                        else:
                            nc.vector.tensor_copy(dstk[0:D, cs], pt[:])
                        nc.vector.tensor_sub(
                            dstk[D : 2 * D, cs], pt[:], dstk[0:D, cs].bitcast(F32)
                        )
                        tv = targ_d[b, c * PB : (c + 1) * PB].rearrange(
                            "(p one) -> p one", one=1
                        )
                        nc.sync.dma_start(tpart[c][:, b : b + 1], tv)
                for c in range(NCH):
                    nc.vector.tensor_scalar(
                        out=tneg[c][:], in0=tpart[c][:], scalar1=-1.0,
                        scalar2=None, op0=OP.mult,
                    )

            # ---- main pipeline ----
            with (
                tc.tile_pool(name="sqp", bufs=3) as sqp,
                tc.tile_pool(name="avp", bufs=2) as avp,
                tc.tile_pool(name="evp", bufs=2) as evp,
                tc.tile_pool(name="mscr", bufs=2) as mscrp,
                tc.tile_pool(name="small", bufs=4) as small,
                tc.tile_pool(name="rpp", bufs=2, space="PSUM") as rpp,
                tc.tile_pool(name="spp", bufs=2, space="PSUM") as spp,
            ):

                def s1_alloc(b):
                    av = [
                        avp.tile([PB, T], F32, tag=f"av{c}", name=f"av{c}")
                        for c in range(NCH)
                    ]
                    mx2 = [
                        small.tile([PB, NJT], F32, tag=f"mx2{c}", name=f"mx2{c}")
                        for c in range(NCH)
                    ]
                    return av, mx2

                def _bias_emit(b, c, mx2):
                    """exp bias = 0.5 * min_t S; emitted per chunk as soon as
                    that chunk's last evac partial lands (shortens the
                    evac->bias->exp->numerator boundary chain)."""
                    scr = small.tile([PB, NJT], F32, tag=f"bsc{c}", name=f"bsc{c}")
                    bias = small.tile([PB, 1], F32, tag=f"bias{c}", name=f"bias{c}")
                    nc.vector.tensor_scalar(
                        out=scr[:], in0=mx2[c][:], scalar1=0.5, scalar2=None,
                        op0=OP.mult, op1=OP.min, accum_out=bias[:],
                    )
                    return bias

                def s1_round(b, jt, av, mx2, biases):
                    """per-jt chain: Z -> sq -> cumsum -> evac(+row min)."""
                    js = slice(jt * PT, (jt + 1) * PT)
                    sqs = []
                    for c in range(NCH):
                        cs = slice(b * N + c * PB, b * N + (c + 1) * PB)
                        rp = rpp.tile([PB, PT], F32, tag="rp", name="rp")
                        for h in range(NMM):
                            lo_ = jt * PT + h * MT
                            nc.tensor.matmul(
                                rp[:, h * MT : (h + 1) * MT],
                                dstk[:, cs], wrep[:, lo_ : lo_ + MT],
                            )
                        sq = sqp.tile([PB, PT], F32R, tag=f"sq{c}", name=f"sq{c}")
                        nc.scalar.activation(
                            sq[:], rp[:], AF.Square,
                            bias=tneg[c][:, b : b + 1], scale=1.0,
                        )
                        sqs.append(sq)
                    for c in range(NCH):
                        sp = spp.tile([PB, PT], F32, tag="sp", name="sp")
                        for h in range(NMM):
                            hsl = slice(h * MT, (h + 1) * MT)
                            nc.tensor.matmul(
                                sp[:, hsl], utri[:], sqs[c][:, hsl],
                                start=True, stop=(c == 0),
                            )
                            if c == 1:
                                nc.tensor.matmul(
                                    sp[:, hsl], onesm[:], sqs[0][:, hsl],
                                    start=False, stop=True,
                                )
                        if _evac_on_act(b, jt, c):
                            # offload: Act copies PSUM->SBUF; the row min runs
                            # on VectorE from SBUF (all-SBUF TensorScalar gets
                            # the 2x DVE rate; TensorReduce does not)
                            nc.scalar.activation(av[c][:, js], sp[:], AF.Copy)
                            ms = mscrp.tile([PB, PT], F32, tag="ms", name="ms")
                            nc.vector.tensor_scalar(
                                out=ms[:], in0=av[c][:, js], scalar1=1.0,
                                scalar2=None, op0=OP.mult, op1=OP.min,
                                accum_out=mx2[c][:, jt : jt + 1],
                            )
                        else:
                            nc.vector.tensor_scalar(
                                out=av[c][:, js], in0=sp[:], scalar1=1.0,
                                scalar2=None, op0=OP.mult, op1=OP.min,
                                accum_out=mx2[c][:, jt : jt + 1],
                            )
                        if jt == NJT - 1:
                            biases.append(_bias_emit(b, c, mx2))

                def s2_alloc(b):
                    den4 = [
                        small.tile([PB, NJT], F32, tag=f"den4{c}", name=f"den4{c}")
                        for c in range(NCH)
                    ]
                    num4 = [
                        small.tile([PB, NJT], F32, tag=f"num4{c}", name=f"num4{c}")
                        for c in range(NCH)
                    ]
                    return den4, num4

                def s2_round(b, jt, av, biases, den4, num4):
                    """exp (accum den) -> Z recompute -> fused (Z-t)*E."""
                    js = slice(jt * PT, (jt + 1) * PT)
                    for c in range(NCH):
                        cs = slice(b * N + c * PB, b * N + (c + 1) * PB)
                        ev = evp.tile([PB, PT], F32, tag=f"E{c}", name=f"E{c}")
                        nc.scalar.activation(
                            ev[:], av[c][:, js], AF.Exp,
                            bias=biases[c][:], scale=-0.5,
                            accum_out=den4[c][:, jt : jt + 1],
                        )
                        rp2 = spp.tile([PB, PT], F32, tag="sp", name="rp2")
                        for h in range(NMM):
                            lo_ = jt * PT + h * MT
                            nc.tensor.matmul(
                                rp2[:, h * MT : (h + 1) * MT],
                                dstk[:, cs], wrep[:, lo_ : lo_ + MT],
                            )
                        ns = mscrp.tile([PB, PT], F32, tag="ns", name="ns")
                        nc.vector.scalar_tensor_tensor(
                            out=ns[:], in0=rp2[:],
                            scalar=tpart[c][:, b : b + 1], in1=ev[:],
                            op0=OP.subtract, op1=OP.mult,
                            accum_out=num4[c][:, jt : jt + 1],
                        )

                def s2_finish(b, den4, num4):
                    for c in range(NCH):
                        nc.vector.tensor_reduce(
                            num[c][:, b : b + 1], num4[c][:],
                            axis=mybir.AxisListType.X, op=OP.add,
                        )
                        nc.vector.tensor_reduce(
                            den[c][:, b : b + 1], den4[c][:],
                            axis=mybir.AxisListType.X, op=OP.add,
                        )

                # modulo-scheduled pipeline: per-jt rounds interleave batch b's
                # stage-1 chain with batch b-1's stage-2 chain so each engine's
                # in-order stream always has ready work at the front.
                # s2_finish(b) is deliberately emitted one iteration later
                # (mid-round): den/num are only read by the finals, and
                # emitting the reduces right after the last round would stall
                # VectorE on the exp->den chain of the final jt tile.
                prev = None
                fin = None
                for b in range(BPC):
                    av, mx2 = s1_alloc(b)
                    biases = []
                    if prev is not None:
                        pb, pav, pbias, pden4, pnum4 = prev
                    for jt in range(NJT):
                        if prev is not None:
                            s2_round(pb, jt, pav, pbias, pden4, pnum4)
                        s1_round(b, jt, av, mx2, biases)
                        if jt == 1 and fin is not None:
                            s2_finish(*fin)
                            fin = None
                    if prev is not None:
                        fin = (pb, pden4, pnum4)
                    den4, num4 = s2_alloc(b)
                    prev = (b, av, biases, den4, num4)
                pb, pav, pbias, pden4, pnum4 = prev
                for jt in range(NJT):
                    s2_round(pb, jt, pav, pbias, pden4, pnum4)
                    if jt == 1 and fin is not None:
                        s2_finish(*fin)
                        fin = None
                s2_finish(pb, pden4, pnum4)

                # finals: out = targets + num/den
                for c in range(NCH):
                    rec = small.tile([PB, BPC], F32, tag=f"rec{c}", name=f"rec{c}")
                    prod = small.tile([PB, BPC], F32, tag=f"prod{c}", name=f"prod{c}")
                    outv = small.tile([PB, BPC], F32, tag=f"outv{c}", name=f"outv{c}")
                    nc.vector.reciprocal(rec[:], den[c][:])
                    nc.vector.tensor_mul(prod[:], num[c][:], rec[:])
                    nc.vector.tensor_add(outv[:], tpart[c][:], prod[:])
                    ov = out_d[:, c * PB : (c + 1) * PB].rearrange("b p -> p b")
                    nc.sync.dma_start(ov, outv[:])

    nc.compile()
    return nc


def _get_nc():
    global _cached_nc
    if _cached_nc is None:
        _cached_nc = _build()
    return _cached_nc


_cached_runner = None


def _get_runner():
    """Build once: a cached jax.jit shard_map over the 8 NeuronCores.

    run_bass_kernel_spmd/run_bass_via_pjrt construct a fresh jax.jit closure
    per call (full retrace); caching the callable keeps repeat calls cheap.
    """
    global _cached_runner
    if _cached_runner is None:
        import jax
        from jax.sharding import Mesh, PartitionSpec
        from concourse import bass2jax
        from concourse.bass2jax import _bass_exec_p, partition_id_tensor
        import concourse.mybir as mybir

        try:
            from jax.experimental.shard_map import shard_map
        except ImportError:
            from jax.shard_map import shard_map

        bass2jax.install_neuronx_cc_hook()
        nc = _get_nc()
        partition_name = (
            nc.partition_id_tensor.name if nc.partition_id_tensor else None
        )
        in_names, out_names, out_avals, zero_outs = [], [], [], []
        for alloc in nc.m.functions[0].allocations:
            if not isinstance(alloc, mybir.MemoryLocationSet):
                continue
            name = alloc.memorylocations[0].name
            if alloc.kind == "ExternalInput":
                if name != partition_name:
                    in_names.append(name)
            elif alloc.kind == "ExternalOutput":
                out_names.append(name)
                shape = tuple(alloc.tensor_shape)
                dtype = mybir.dt.np(alloc.dtype)
                out_avals.append(jax.core.ShapedArray(shape, dtype))
                zero_outs.append(np.zeros((NCORES * shape[0], *shape[1:]), dtype))
        n_params = len(in_names)
        all_names = list(in_names) + list(out_names)
        if partition_name is not None:
            all_names.append(partition_name)
        donate = tuple(range(n_params, n_params + len(out_names)))

        def _body(*args):
            operands = list(args)
            if partition_name is not None:
                operands.append(partition_id_tensor())
            return tuple(
                _bass_exec_p.bind(
                    *operands,
                    out_avals=tuple(out_avals),
                    in_names=tuple(all_names),
                    out_names=tuple(out_names),
                    lowering_input_output_aliases=(),
                    sim_require_finite=True,
                    sim_require_nnan=True,
                    nc=nc,
                )
            )

        devices = jax.devices()[:NCORES]
        mesh = Mesh(np.asarray(devices), ("core",))
        in_specs = tuple(
            PartitionSpec() if name == "task_pool" else PartitionSpec("core")
            for name in in_names
        ) + (PartitionSpec("core"),) * len(out_names)
        sharded = jax.jit(
            shard_map(
                _body,
                mesh=mesh,
                in_specs=in_specs,
                out_specs=(PartitionSpec("core"),) * len(out_names),
                check_rep=False,
            ),
            donate_argnums=donate,
            keep_unused=True,
        )
        _cached_runner = (sharded, in_names, out_names, out_avals, zero_outs)
    return _cached_runner


def _kernel_fallback(data, targets, tp):
    """Robust path via the stock SPMD runner (fresh jit each call)."""
    from concourse.bass_utils import run_bass_kernel_spmd

    nc = _get_nc()
    in_maps = [
        {
            "data": data[i * BPC : (i + 1) * BPC],
            "targets": targets[i * BPC : (i + 1) * BPC],
            "task_pool": tp,
        }
        for i in range(NCORES)
    ]
    res = run_bass_kernel_spmd(nc, in_maps, core_ids=list(range(NCORES)))
    return np.concatenate([r["out"] for r in res.results], axis=0)


def kernel(data, targets, task_pool, **_):
    data = np.ascontiguousarray(np.asarray(data, np.float32))
    targets = np.ascontiguousarray(np.asarray(targets, np.float32))
    tp = np.ascontiguousarray(np.asarray(task_pool, np.float32).reshape(T, D))

    try:
        sharded, in_names, out_names, out_avals, zero_outs = _get_runner()
        full = {
            "data": data.reshape(NCORES * BPC, N, D),
            "targets": targets.reshape(NCORES * BPC, N),
            "task_pool": tp,
        }
        args = [full[name] for name in in_names]
        args += [np.zeros_like(z) for z in zero_outs]
        outs = sharded(*args)
        out = np.asarray(outs[out_names.index("out")])
        return out.reshape(B, N)
    except Exception:
        return _kernel_fallback(data, targets, tp)



# revision 35
# speedup vs baseline: 1.4300x; 1.0282x over previous
"""DiscreteMMSE Trainium2 Bass kernel.

Math (per batch row b):
  Z = data[b] @ W                      [N, T]   (W = squeeze(task_pool).T)
  sq = (Z - targets[b][:, None])^2     [N, T]
  S[i] = sum_{n<i} sq[n]               (strict cumsum over N; S[0] = 0)
  E = exp(-0.5*(S - min_t S))          (softmax-stable weights)
  out[b, i] = targets[b, i] + (sum_t E[i]*(Z-t)[i]) / (sum_t E[i])

Identical to the reference softmax-posterior MMSE prediction: the Gaussian
log-pdf constant and row-wise shifts cancel in the softmax, and
pred = sum_t post*Z = t + sum_t post*(Z-t). Row 0 (uniform prior over
tasks) falls out of the strict cumsum.

Layout per NeuronCore (pure data parallel over B: 8 rows each, no
collectives). N=256 rows on partitions as two 128-row chunks; T=4096 on
the free dim. Engine budget per (b,jt) round is balanced across all four
compute engines:
  - TensorE: Z via ONE 128-contraction f32r matmul per 512-slice with
    lhsT=[d_hi; d_lo] (hi/lo split of data.T) and rhs=[W; W] (full fp32
    bits bitcast to f32r; the PE's 12-bit mantissa drop on read is
    compensated by the lo plane of data, and W's truncation noise is
    ~1e-3 per Z entry -- well within tolerance). Strict cumsum over N via
    triangular-ones f32r matmuls (chunk1 adds ones.T@sq0), reading the
    fp32 sq tile bitcast as f32r (truncation noise ~0.07 nats on logits).
    Z is recomputed for stage 2 (cheaper than buffering it in SBUF).
  - ScalarE (Act): sq = Square(Z + bias) with per-partition bias=-targets
    straight out of PSUM; E = ONE big [128, 4096] Exp per (b,chunk) with
    scale=-0.5, bias=0.5*min_t S, accum_out = denominator. A minority of
    PSUM->SBUF cumsum evacuations also run here (Copy) to offload VectorE.
  - VectorE (DVE): most cumsum evacuations PSUM->SBUF fused with the
    running row-min (tensor_scalar accum min); numerator via ONE fused
    scalar_tensor_tensor: (Z_psum - t)*E with accum_out = running sum.
  - GpSimd (Pool): row-min for the Act-evacuated tiles (SBUF only; GpSimd
    has no PSUM port).
  - modulo-scheduled emission: engines execute their instruction streams
    IN ORDER, so per-jt rounds interleave batch b's stage-1 chain
    (Z->sq->cumsum->evac) with batch b-1's stage-2 chain (Zre->E*(Z-t));
    the big Exp for b runs between the round groups.
"""

import numpy as np

B, N, D, T = 64, 256, 64, 4096
NCORES = 8
BPC = B // NCORES  # batch rows per core
NCH = 2            # partition chunks of N
PB = 128           # partitions per chunk
PT = 1024          # psum tile free size (2 banks)
MT = 512           # matmul moving free size (1 bank)
NJT = T // PT      # psum tiles per chunk row
NMM = PT // MT     # matmuls per psum tile

_cached_nc = None


def _evac_on_act(b, jt, c):
    """Which cumsum evacuations run on ScalarE+GpSimd instead of VectorE.

    ~9 of 64 rebalances DVE (evac+numerator) against Act (square+exp).
    Only early jts: the row-min of late jts gates the exp bias."""
    return c == 0 and (jt == 1 or (jt == 2 and b == 3))


def _build():
    import concourse.bacc as bacc
    import concourse.mybir as mybir
    import concourse.tile as tile
    from concourse import masks

    F32 = mybir.dt.float32
    F32R = mybir.dt.float32r
    AF = mybir.ActivationFunctionType
    OP = mybir.AluOpType

    nc = bacc.Bacc("TRN2", debug=False)
    data_d = nc.dram_tensor("data", (BPC, N, D), F32, kind="ExternalInput")
    targ_d = nc.dram_tensor("targets", (BPC, N), F32, kind="ExternalInput")
    pool_d = nc.dram_tensor("task_pool", (T, D), F32, kind="ExternalInput")
    out_d = nc.dram_tensor("out", (BPC, N), F32, kind="ExternalOutput")

    with tile.TileContext(nc) as tc:
        with tc.tile_pool(name="const", bufs=1) as const:
            utri = const.tile([PB, PB], F32R)     # strictly-upper ones (lhsT)
            onesm = const.tile([PB, PB], F32R)    # all-ones
            wrep = const.tile([PB, T], F32R)      # [W ; W] full fp32 bits
            dstk = const.tile([PB, BPC * N], F32R)  # [data.T hi ; data.T lo]
            tpart = [const.tile([PB, BPC], F32, name=f"tpart{c}", tag=f"tpart{c}") for c in range(NCH)]
            tneg = [const.tile([PB, BPC], F32, name=f"tneg{c}", tag=f"tneg{c}") for c in range(NCH)]
            den = [const.tile([PB, BPC], F32, name=f"den{c}", tag=f"den{c}") for c in range(NCH)]
            num = [const.tile([PB, BPC], F32, name=f"num{c}", tag=f"num{c}") for c in range(NCH)]


            nc.any.memset(onesm[:].bitcast(F32), 1.0)

            # ---- setup: transpose task_pool and data into lhsT layouts ----
            with (
                tc.tile_pool(name="ld", bufs=1) as ld,
                tc.tile_pool(name="tps", bufs=4, space="PSUM") as tps,
            ):
                # kick off all input DMAs first so they overlap mask setup
                wbig = ld.tile([PB, (T // PB) * D], F32, tag="wbig", name="wbig")
                NK = T // PB
                KC = NK // 8  # chunk the load so transposes overlap the DMA
                # data + targets ride the Activation DMA ring so they overlap
                # the task-pool load on the SP ring
                dball = ld.tile([PB, BPC * NCH * D], F32, tag="dball", name="dball")
                nc.scalar.dma_start(
                    dball[:].rearrange("p (b c d) -> p b c d", c=NCH, d=D),
                    data_d[:].rearrange("b (c p) d -> p b c d", p=PB),
                )
                for c in range(NCH):
                    nc.scalar.dma_start(
                        tpart[c][:],
                        targ_d[:, c * PB : (c + 1) * PB].rearrange("b p -> p b"),
                    )
                for q in range(8):
                    nc.sync.dma_start(
                        wbig[:, q * KC * D : (q + 1) * KC * D].rearrange(
                            "p (k d) -> p k d", d=D
                        ),
                        pool_d[q * KC * PB : (q + 1) * KC * PB].rearrange(
                            "(k p) d -> p k d", p=PB
                        ),
                    )
                ident = ld.tile([PB, PB], F32, tag="ident", name="ident")
                masks.make_identity(nc, ident[:])
                utri_f = ld.tile([PB, PB], F32, tag="utri_f", name="utri_f")
                masks.make_upper_triangular(nc, utri_f[:], 1.0, diag=False)
                nc.vector.tensor_copy(utri[:], utri_f[:])
                for k in range(NK):
                    pt = tps.tile([D, PB], F32, tag="pt", name="pt")
                    nc.tensor.transpose(pt[:], wbig[:, k * D : (k + 1) * D], ident[:])
                    # split the PSUM->SBUF evacuations across Act and DVE
                    if k % 2 == 0:
                        nc.scalar.activation(
                            wrep[0:D, k * PB : (k + 1) * PB], pt[:], AF.Copy
                        )
                    else:
                        nc.vector.tensor_copy(
                            wrep[0:D, k * PB : (k + 1) * PB], pt[:]
                        )
                # duplicate the f32r-rounded W into the low 64 partitions
                # (GpSimd: SBUF-only copy on the otherwise idle engine)
                nc.gpsimd.tensor_copy(wrep[D : 2 * D, :], wrep[0:D, :])
                for b in range(BPC):
                    for c in range(NCH):
                        cs = slice(b * N + c * PB, b * N + (c + 1) * PB)
                        pt = tps.tile([D, PB], F32, tag="pt", name="pt")
                        nc.tensor.transpose(
                            pt[:],
                            dball[:, (b * NCH + c) * D : (b * NCH + c + 1) * D],
                            ident[:],
                        )
                        # hi: f32r-rounding convert copy; lo: exact fp32 rest
                        if c == 0:
                            nc.scalar.activation(dstk[0:D, cs], pt[:], AF.Copy)
                        else:
                            nc.vector.tensor_copy(dstk[0:D, cs], pt[:])
                        nc.vector.tensor_sub(
                            dstk[D : 2 * D, cs], pt[:], dstk[0:D, cs].bitcast(F32)
                        )
                for c in range(NCH):
                    nc.vector.tensor_scalar(
                        out=tneg[c][:], in0=tpart[c][:], scalar1=-1.0,
                        scalar2=None, op0=OP.mult,
                    )

            # ---- main pipeline ----
            with (
                tc.tile_pool(name="sqp", bufs=3) as sqp,
                tc.tile_pool(name="avp", bufs=2) as avp,
                tc.tile_pool(name="evp", bufs=2) as evp,
                tc.tile_pool(name="mscr", bufs=2) as mscrp,
                tc.tile_pool(name="small", bufs=4) as small,
                tc.tile_pool(name="rpp", bufs=2, space="PSUM") as rpp,
                tc.tile_pool(name="spp", bufs=2, space="PSUM") as spp,
            ):

                def s1_alloc(b):
                    av = [
                        avp.tile([PB, T], F32, tag=f"av{c}", name=f"av{c}")
                        for c in range(NCH)
                    ]
                    mx2 = [
                        small.tile([PB, NJT], F32, tag=f"mx2{c}", name=f"mx2{c}")
                        for c in range(NCH)
                    ]
                    return av, mx2

                def _bias_emit(b, c, mx2):
                    """exp bias = 0.5 * min_t S; emitted per chunk as soon as
                    that chunk's last evac partial lands (shortens the
                    evac->bias->exp->numerator boundary chain)."""
                    scr = small.tile([PB, NJT], F32, tag=f"bsc{c}", name=f"bsc{c}")
                    bias = small.tile([PB, 1], F32, tag=f"bias{c}", name=f"bias{c}")
                    nc.vector.tensor_scalar(
                        out=scr[:], in0=mx2[c][:], scalar1=0.5, scalar2=None,
                        op0=OP.mult, op1=OP.min, accum_out=bias[:],
                    )
                    return bias

                def s1_round(b, jt, av, mx2, biases):
                    """per-jt chain: Z -> sq -> cumsum -> evac(+row min)."""
                    js = slice(jt * PT, (jt + 1) * PT)
                    sqs = []
                    for c in range(NCH):
                        cs = slice(b * N + c * PB, b * N + (c + 1) * PB)
                        rp = rpp.tile([PB, PT], F32, tag="rp", name="rp")
                        for h in range(NMM):
                            lo_ = jt * PT + h * MT
                            nc.tensor.matmul(
                                rp[:, h * MT : (h + 1) * MT],
                                dstk[:, cs], wrep[:, lo_ : lo_ + MT],
                            )
                        sq = sqp.tile([PB, PT], F32R, tag=f"sq{c}", name=f"sq{c}")
                        nc.scalar.activation(
                            sq[:], rp[:], AF.Square,
                            bias=tneg[c][:, b : b + 1], scale=1.0,
                        )
                        sqs.append(sq)
                    for c in range(NCH):
                        sp = spp.tile([PB, PT], F32, tag="sp", name="sp")
                        for h in range(NMM):
                            hsl = slice(h * MT, (h + 1) * MT)
                            nc.tensor.matmul(
                                sp[:, hsl], utri[:], sqs[c][:, hsl],
                                start=True, stop=(c == 0),
                            )
                            if c == 1:
                                nc.tensor.matmul(
                                    sp[:, hsl], onesm[:], sqs[0][:, hsl],
                                    start=False, stop=True,
                                )
                        if _evac_on_act(b, jt, c):
                            # offload: Act copies PSUM->SBUF; the row min runs
                            # on VectorE from SBUF (all-SBUF TensorScalar gets
                            # the 2x DVE rate; TensorReduce does not)
                            nc.scalar.activation(av[c][:, js], sp[:], AF.Copy)
                            ms = mscrp.tile([PB, PT], F32, tag="ms", name="ms")
                            nc.vector.tensor_scalar(
                                out=ms[:], in0=av[c][:, js], scalar1=1.0,
                                scalar2=None, op0=OP.mult, op1=OP.min,
                                accum_out=mx2[c][:, jt : jt + 1],
                            )
                        else:
                            nc.vector.tensor_scalar(
                                out=av[c][:, js], in0=sp[:], scalar1=1.0,
                                scalar2=None, op0=OP.mult, op1=OP.min,
                                accum_out=mx2[c][:, jt : jt + 1],
                            )
                        if jt == NJT - 1:
                            biases.append(_bias_emit(b, c, mx2))

                def s2_alloc(b):
                    den4 = [
                        small.tile([PB, NJT], F32, tag=f"den4{c}", name=f"den4{c}")
                        for c in range(NCH)
                    ]
                    num4 = [
                        small.tile([PB, NJT], F32, tag=f"num4{c}", name=f"num4{c}")
                        for c in range(NCH)
                    ]
                    return den4, num4

                def s2_round(b, jt, av, biases, den4, num4):
                    """exp (accum den) -> Z recompute -> fused (Z-t)*E."""
                    js = slice(jt * PT, (jt + 1) * PT)
                    for c in range(NCH):
                        cs = slice(b * N + c * PB, b * N + (c + 1) * PB)
                        ev = evp.tile([PB, PT], F32, tag=f"E{c}", name=f"E{c}")
                        nc.scalar.activation(
                            ev[:], av[c][:, js], AF.Exp,
                            bias=biases[c][:], scale=-0.5,
                            accum_out=den4[c][:, jt : jt + 1],
                        )
                        rp2 = spp.tile([PB, PT], F32, tag="sp", name="rp2")
                        for h in range(NMM):
                            lo_ = jt * PT + h * MT
                            nc.tensor.matmul(
                                rp2[:, h * MT : (h + 1) * MT],
                                dstk[:, cs], wrep[:, lo_ : lo_ + MT],
                            )
                        ns = mscrp.tile([PB, PT], F32, tag="ns", name="ns")
                        nc.vector.scalar_tensor_tensor(
                            out=ns[:], in0=rp2[:],
                            scalar=tpart[c][:, b : b + 1], in1=ev[:],
                            op0=OP.subtract, op1=OP.mult,
                            accum_out=num4[c][:, jt : jt + 1],
                        )

                def s2_finish(b, den4, num4):
                    for c in range(NCH):
                        nc.vector.tensor_reduce(
                            num[c][:, b : b + 1], num4[c][:],
                            axis=mybir.AxisListType.X, op=OP.add,
                        )
                        nc.vector.tensor_reduce(
                            den[c][:, b : b + 1], den4[c][:],
                            axis=mybir.AxisListType.X, op=OP.add,
                        )

                # modulo-scheduled pipeline: per-jt rounds interleave batch b's
                # stage-1 chain with batch b-1's stage-2 chain so each engine's
                # in-order stream always has ready work at the front.
                # s2_finish(b) is deliberately emitted one iteration later
                # (mid-round): den/num are only read by the finals, and
                # emitting the reduces right after the last round would stall
                # VectorE on the exp->den chain of the final jt tile.
                prev = None
                fin = None
                for b in range(BPC):
                    av, mx2 = s1_alloc(b)
                    biases = []
                    if prev is not None:
                        pb, pav, pbias, pden4, pnum4 = prev
                    for jt in range(NJT):
                        if prev is not None:
                            s2_round(pb, jt, pav, pbias, pden4, pnum4)
                        s1_round(b, jt, av, mx2, biases)
                        if jt == 1 and fin is not None:
                            s2_finish(*fin)
                            fin = None
                    if prev is not None:
                        fin = (pb, pden4, pnum4)
                    den4, num4 = s2_alloc(b)
                    prev = (b, av, biases, den4, num4)
                pb, pav, pbias, pden4, pnum4 = prev
                for jt in range(NJT):
                    s2_round(pb, jt, pav, pbias, pden4, pnum4)
                    if jt == 1 and fin is not None:
                        s2_finish(*fin)
                        fin = None
                s2_finish(pb, pden4, pnum4)

                # finals: out = targets + num/den
                for c in range(NCH):
                    rec = small.tile([PB, BPC], F32, tag=f"rec{c}", name=f"rec{c}")
                    prod = small.tile([PB, BPC], F32, tag=f"prod{c}", name=f"prod{c}")
                    outv = small.tile([PB, BPC], F32, tag=f"outv{c}", name=f"outv{c}")
                    nc.vector.reciprocal(rec[:], den[c][:])
                    nc.vector.tensor_mul(prod[:], num[c][:], rec[:])
                    nc.vector.tensor_add(outv[:], tpart[c][:], prod[:])
                    ov = out_d[:, c * PB : (c + 1) * PB].rearrange("b p -> p b")
                    nc.sync.dma_start(ov, outv[:])

    nc.compile()
    return nc


def _get_nc():
    global _cached_nc
    if _cached_nc is None:
        _cached_nc = _build()
    return _cached_nc


_cached_runner = None


def _get_runner():
    """Build once: a cached jax.jit shard_map over the 8 NeuronCores.

    run_bass_kernel_spmd/run_bass_via_pjrt construct a fresh jax.jit closure
    per call (full retrace); caching the callable keeps repeat calls cheap.
    """
    global _cached_runner
    if _cached_runner is None:
        import jax
        from jax.sharding import Mesh, PartitionSpec
        from concourse import bass2jax
        from concourse.bass2jax import _bass_exec_p, partition_id_tensor
        import concourse.mybir as mybir

        try:
            from jax.experimental.shard_map import shard_map
        except ImportError:
            from jax.shard_map import shard_map

        bass2jax.install_neuronx_cc_hook()
        nc = _get_nc()
        partition_name = (
            nc.partition_id_tensor.name if nc.partition_id_tensor else None
        )
        in_names, out_names, out_avals, zero_outs = [], [], [], []
        for alloc in nc.m.functions[0].allocations:
            if not isinstance(alloc, mybir.MemoryLocationSet):
                continue
            name = alloc.memorylocations[0].name
            if alloc.kind == "ExternalInput":
                if name != partition_name:
                    in_names.append(name)
            elif alloc.kind == "ExternalOutput":
                out_names.append(name)
                shape = tuple(alloc.tensor_shape)
                dtype = mybir.dt.np(alloc.dtype)
                out_avals.append(jax.core.ShapedArray(shape, dtype))
                zero_outs.append(np.zeros((NCORES * shape[0], *shape[1:]), dtype))
        n_params = len(in_names)
        all_names = list(in_names) + list(out_names)
        if partition_name is not None:
            all_names.append(partition_name)
        donate = tuple(range(n_params, n_params + len(out_names)))

        def _body(*args):
            operands = list(args)
            if partition_name is not None:
                operands.append(partition_id_tensor())
            return tuple(
                _bass_exec_p.bind(
                    *operands,
                    out_avals=tuple(out_avals),
                    in_names=tuple(all_names),
                    out_names=tuple(out_names),
                    lowering_input_output_aliases=(),
                    sim_require_finite=True,
                    sim_require_nnan=True,
                    nc=nc,
                )
            )

        devices = jax.devices()[:NCORES]
        mesh = Mesh(np.asarray(devices), ("core",))
        in_specs = tuple(
            PartitionSpec() if name == "task_pool" else PartitionSpec("core")
            for name in in_names
        ) + (PartitionSpec("core"),) * len(out_names)
        sharded = jax.jit(
            shard_map(
                _body,
                mesh=mesh,
                in_specs=in_specs,
                out_specs=(PartitionSpec("core"),) * len(out_names),
                check_rep=False,
            ),
            donate_argnums=donate,
            keep_unused=True,
        )
        _cached_runner = (sharded, in_names, out_names, out_avals, zero_outs)
    return _cached_runner


def _kernel_fallback(data, targets, tp):
    """Robust path via the stock SPMD runner (fresh jit each call)."""
    from concourse.bass_utils import run_bass_kernel_spmd

    nc = _get_nc()
    in_maps = [
        {
            "data": data[i * BPC : (i + 1) * BPC],
            "targets": targets[i * BPC : (i + 1) * BPC],
            "task_pool": tp,
        }
        for i in range(NCORES)
    ]
    res = run_bass_kernel_spmd(nc, in_maps, core_ids=list(range(NCORES)))
    return np.concatenate([r["out"] for r in res.results], axis=0)


def kernel(data, targets, task_pool, **_):
    data = np.ascontiguousarray(np.asarray(data, np.float32))
    targets = np.ascontiguousarray(np.asarray(targets, np.float32))
    tp = np.ascontiguousarray(np.asarray(task_pool, np.float32).reshape(T, D))

    try:
        sharded, in_names, out_names, out_avals, zero_outs = _get_runner()
        full = {
            "data": data.reshape(NCORES * BPC, N, D),
            "targets": targets.reshape(NCORES * BPC, N),
            "task_pool": tp,
        }
        args = [full[name] for name in in_names]
        args += [np.zeros_like(z) for z in zero_outs]
        outs = sharded(*args)
        out = np.asarray(outs[out_names.index("out")])
        return out.reshape(B, N)
    except Exception:
        return _kernel_fallback(data, targets, tp)

